# revision 26
# baseline (speedup 1.0000x reference)
"""Cross-attention Trainium2 kernel (8-core SPMD, batch-parallel).

Reference computation (B=16, Lq=4096, Lkv=77, D=1024, C=768):
    q = x@Wq + bq; k = y@Wk + bk; v = y@Wv + bv
    attn = softmax((q @ k^T) / sqrt(128));  out = (attn @ v) @ Wo + bo

Because Lkv=77 << D=1024, we use associativity to avoid materializing q:
    ktT = (y@Wk + bk)^T            [D, 77]   (per batch)
    vtT = (y@Wv + bv)^T            [D, 77]
    Cb  = Wq @ ktT                 [D, 77]   -> scores^T = Cb^T @ x^T + d
    d   = bq^T @ ktT               [77]      (row constant)
    E   = vtT^T @ Wo + 1*bo^T      [77, D]   -> out = attn @ E  (exact,
                                               since attn rows sum to 1)
This reduces FLOPs ~10x (299 -> 30 GFLOP) and makes the kernel HBM-bound
(~82 MB HBM traffic per core: x 33.5 read + out 33.5 write + weights 14.6).
Softmax is computed without max-subtraction (logits ~ N(0, 2.8^2), far
from fp32/bf16 overflow), unnormalized exp^T goes through the attn@E
matmul and the 1/rowsum is applied at the end as a per-partition scalar.

Implementation notes (measured on silicon):
- x must be consumed transposed (d on partitions). DMA-xbar transposes
  serialize ~1.2us/call on the SP sequencer (512 calls -> +700us), so all
  transposes run on the TensorEngine (is_transpose matmul vs identity,
  4 blocks batched per PSUM bank) with DVE copies back to SBUF.
- All HBM DMA goes through SWDGE (gpsimd), which casts f32->bf16 inline.
  Tokens are permuted so each partition holds 2 consecutive DRAM rows
  ("(c p t) d" APs, t=2): 8KB-contiguous descriptors -> 4KB write packets
  (instead of 2KB), relieving the SDMA packet-rate limit. The same
  permuted order is used for xT blocks / o_sb / the out-DMA AP, so it
  cancels out end-to-end.
- fp32r matmuls measure ~bf16 precision on TRN2 (single rounded pass),
  so bf16 storage is used throughout (fp32 PSUM accumulation).
- Biases fold in exactly when nonzero: bk/bv as per-partition DVE
  scalar-adds, bq via an extra k~T x bq matmul into the exp() bias,
  bo via a K=1 ones-row matmul accumulated into E.

HW exec ~234us/NEFF (8 cores SPMD); end-to-end rel err ~7.6e-3 (L2).
"""
import sys

for _p in ("/opt/trn_rl_repo",):
    if _p not in sys.path:
        sys.path.insert(0, _p)

import numpy as np
import concourse.bass as bass
from concourse import mybir, tile, bacc, masks
from concourse.bass_utils import run_bass_kernel_spmd

N_CORES = 8
B, LQ, LKV, D, C = 16, 4096, 77, 1024, 768
BPC = B // N_CORES          # batches per core
TOKT = 512                  # query-token tile
NTILE = LQ // TOKT          # 8 token tiles per batch
DC = D // 128               # 8 chunks of the embed dim
CC = C // 128               # 6 chunks of the cross dim
SCALE = 1.0 / np.sqrt(D // 8)  # 1/sqrt(128), matches reference

BF = mybir.dt.float32 if False else mybir.dt.bfloat16
F32 = mybir.dt.float32

LAST_EXEC_TIME_NS = None
LAST_RESULTS = None


def _build(use_bias: bool):
    nc = bacc.Bacc("TRN2", target_bir_lowering=False, debug=False,
                   num_devices=N_CORES)
    x_d = nc.declare_dram_parameter("x", [BPC, LQ, D], F32, isOutput=False)
    y_d = nc.declare_dram_parameter("y", [BPC, LKV, C], F32, isOutput=False)
    wq_d = nc.declare_dram_parameter("Wq", [D, D], F32, isOutput=False)
    wk_d = nc.declare_dram_parameter("Wk", [C, D], F32, isOutput=False)
    wv_d = nc.declare_dram_parameter("Wv", [C, D], F32, isOutput=False)
    wo_d = nc.declare_dram_parameter("Wo", [D, D], F32, isOutput=False)
    bq_d = nc.declare_dram_parameter("bq", [D], F32, isOutput=False)
    bk_d = nc.declare_dram_parameter("bk", [D], F32, isOutput=False)
    bv_d = nc.declare_dram_parameter("bv", [D], F32, isOutput=False)
    bo_d = nc.declare_dram_parameter("bo", [D], F32, isOutput=False)
    o_d = nc.declare_dram_parameter("out", [BPC, LQ, D], F32, isOutput=True)

    with tile.TileContext(nc) as tc:
        _emit(nc, tc, use_bias, x_d, y_d, wq_d, wk_d, wv_d, wo_d,
              bq_d, bk_d, bv_d, bo_d, o_d)
    nc.compile()
    return nc


def _emit(nc, tc, use_bias, x_d, y_d, wq_d, wk_d, wv_d, wo_d,
          bq_d, bk_d, bv_d, bo_d, o_d):
    from contextlib import ExitStack
    es = ExitStack()
    with es:
        wpool = es.enter_context(tc.tile_pool(name="w", bufs=1))
        bpool = es.enter_context(tc.tile_pool(name="b", bufs=2))
        xpool = es.enter_context(tc.tile_pool(name="xp", bufs=3))
        opool = es.enter_context(tc.tile_pool(name="op", bufs=3))
        pbig = es.enter_context(tc.tile_pool(name="pb", bufs=3, space="PSUM"))
        ptp = es.enter_context(tc.tile_pool(name="pt", bufs=4, space="PSUM"))
        psmall = es.enter_context(tc.tile_pool(name="pskt", bufs=1, space="PSUM"))

        ident = wpool.tile([128, 128], BF, tag="ident")
        masks.make_identity(nc, ident[:])

        # ---- weights to SBUF (cast f32->bf16 in SWDGE DMA) ----
        wk_sb = wpool.tile([128, CC, D], BF, tag="wk")
        nc.gpsimd.dma_start(wk_sb[:], wk_d.ap().rearrange("(c p) e -> p c e", p=128))
        wv_sb = wpool.tile([128, CC, D], BF, tag="wv")
        nc.gpsimd.dma_start(wv_sb[:], wv_d.ap().rearrange("(c p) e -> p c e", p=128))
        wo_sb = wpool.tile([128, DC, D], BF, tag="wo")
        nc.gpsimd.dma_start(wo_sb[:], wo_d.ap().rearrange("(e p) f -> p e f", p=128))
        # WqT via staged natural load + 64 PE-mode transposes
        wqT_sb = wpool.tile([128, DC, D], BF, tag="wqt")
        with tc.tile_pool(name="wstage", bufs=1) as wstage:
            wq_nat = wstage.tile([128, DC, D], BF, tag="wqnat")
            nc.gpsimd.dma_start(wq_nat[:], wq_d.ap().rearrange("(d p) e -> p d e", p=128))
            for ei in range(DC):
                for dh in range(2):
                    pst = ptp.tile([128, 512], BF, tag="pt")
                    for dq in range(4):
                        di = dh * 4 + dq
                        nc.tensor.transpose(
                            pst[:, dq * 128:(dq + 1) * 128],
                            wq_nat[:, di, ei * 128:(ei + 1) * 128], ident[:])
                    nc.vector.tensor_copy(
                        wqT_sb[:, ei, dh * 512:(dh + 1) * 512], pst[:])

        ones_col = wpool.tile([128, 1], BF, tag="onec")
        nc.vector.memset(ones_col[:], 1.0)
        if use_bias:
            bq_bf = wpool.tile([128, DC], BF, tag="bq")
            nc.gpsimd.dma_start(bq_bf[:], bq_d.ap().rearrange("(c p) -> p c", p=128))
            bk_f = wpool.tile([128, DC], F32, tag="bk")
            nc.sync.dma_start(bk_f[:], bk_d.ap().rearrange("(c p) -> p c", p=128))
            bv_f = wpool.tile([128, DC], F32, tag="bv")
            nc.sync.dma_start(bv_f[:], bv_d.ap().rearrange("(c p) -> p c", p=128))
            bo_bf = wpool.tile([1, D], BF, tag="bo")
            nc.gpsimd.dma_start(bo_bf[:], bo_d.ap()[None, :])
            ones_row = wpool.tile([1, 128], BF, tag="oner")
            nc.vector.memset(ones_row[:], 1.0)

        for b in range(BPC):
            # ---- per-batch prep: yT, ktT, vtT, C, E (+ d) ----
            y_nat = bpool.tile([128, C], BF, tag="ynat")
            # zero the pad rows 77..79 (engine APs need 32-aligned partition
            # start, so clear 64..96 and let the DMA overwrite 64..77)
            nc.vector.memset(y_nat[64:96, :], 0.0)
            nc.gpsimd.dma_start(y_nat[0:LKV, :], y_d.ap()[b])
            yT = bpool.tile([128, CC, 80], BF, tag="yt")
            for ci in range(CC):
                pst = ptp.tile([128, 512], BF, tag="pt")
                nc.tensor.transpose(pst[:, 0:80],
                                    y_nat[0:80, ci * 128:(ci + 1) * 128],
                                    ident[0:80, 0:80])
                nc.vector.tensor_copy(yT[:, ci, :], pst[:, 0:80])

            ktT = bpool.tile([128, DC, LKV], BF, tag="ktt")
            vtT = bpool.tile([128, DC, LKV], BF, tag="vtt")
            for dst, w_sb, bias_tag in ((ktT, wk_sb, "bk"), (vtT, wv_sb, "bv")):
                for ei in range(DC):
                    ps = psmall.tile([128, LKV], F32, tag="pskt")
                    for ci in range(CC):
                        nc.tensor.matmul(ps[:], w_sb[:, ci, ei * 128:(ei + 1) * 128],
                                         yT[:, ci, 0:LKV],
                                         start=(ci == 0), stop=(ci == CC - 1))
                    if use_bias:
                        bsb = bk_f if bias_tag == "bk" else bv_f
                        nc.vector.tensor_scalar_add(dst[:, ei, :], ps[:],
                                                    bsb[:, ei:ei + 1])
                    else:
                        nc.vector.tensor_copy(dst[:, ei, :], ps[:])

            c_sb = bpool.tile([128, DC, LKV], BF, tag="csb")
            for di in range(DC):
                ps = psmall.tile([128, LKV], F32, tag="pskt")
                for ei in range(DC):
                    nc.tensor.matmul(ps[:], wqT_sb[:, ei, di * 128:(di + 1) * 128],
                                     ktT[:, ei, :],
                                     start=(ei == 0), stop=(ei == DC - 1))
                nc.vector.tensor_copy(c_sb[:, di, :], ps[:])

            e_sb = bpool.tile([128, D], BF, tag="esb")
            for fh in range(2):
                ps = pbig.tile([128, 512], F32, tag="ps")
                for ei in range(DC):
                    nc.tensor.matmul(ps[0:LKV, :], vtT[:, ei, :],
                                     wo_sb[:, ei, fh * 512:(fh + 1) * 512],
                                     start=(ei == 0),
                                     stop=(ei == DC - 1) and not use_bias)
                if use_bias:
                    nc.tensor.matmul(ps[0:LKV, :], ones_row[0:1, 0:LKV],
                                     bo_bf[0:1, fh * 512:(fh + 1) * 512],
                                     start=False, stop=True)
                nc.vector.tensor_copy(e_sb[0:LKV, fh * 512:(fh + 1) * 512],
                                      ps[0:LKV, :])

            if use_bias:
                psd = psmall.tile([128, LKV], F32, tag="pskt")
                for ei in range(DC):
                    nc.tensor.matmul(psd[0:LKV, 0:1], ktT[:, ei, :],
                                     bq_bf[:, ei:ei + 1],
                                     start=(ei == 0), stop=(ei == DC - 1))
                d_sb = bpool.tile([128, 1], F32, tag="dsb")
                nc.vector.tensor_scalar_mul(d_sb[0:LKV, :], psd[0:LKV, 0:1], SCALE)

            # ---- per-token-tile pipeline ----
            # Token permutation: partition p holds tokens {c*256+2p+tt} so each
            # DMA descriptor covers 2 consecutive DRAM rows (8KB reads -> 4KB
            # bf16 write packets instead of 2KB). The same permuted order is
            # used in xT blocks, o_sb and the out-DMA AP, so it cancels out.
            for t in range(NTILE):
                x_nat = xpool.tile([128, 2, 2, D], BF, tag="xnat")
                nc.gpsimd.dma_start(
                    x_nat[:],
                    x_d.ap()[b, t * TOKT:(t + 1) * TOKT, :]
                    .rearrange("(c p t) d -> p c t d", p=128, t=2))
                xT = xpool.tile([128, DC, TOKT], BF, tag="xt")
                for di in range(DC):
                    pst = ptp.tile([128, TOKT], BF, tag="pt")
                    for j in range(TOKT // 128):
                        nc.tensor.transpose(
                            pst[:, j * 128:(j + 1) * 128],
                            x_nat[:, j // 2, j % 2, di * 128:(di + 1) * 128],
                            ident[:])
                    nc.vector.tensor_copy(xT[:, di, :], pst[:])

                ps_s = pbig.tile([128, TOKT], F32, tag="ps")
                for di in range(DC):
                    nc.tensor.matmul(ps_s[0:LKV, :], c_sb[:, di, :], xT[:, di, :],
                                     start=(di == 0), stop=(di == DC - 1))
                expT = xpool.tile([128, TOKT], BF, tag="expt")
                nc.scalar.activation(
                    expT[0:LKV, :], ps_s[0:LKV, :],
                    mybir.ActivationFunctionType.Exp,
                    bias=(d_sb[0:LKV, :] if use_bias else 0.0), scale=SCALE)

                ps_sum = psmall.tile([128, LKV], F32, tag="pskt")
                for tc4 in range(TOKT // 128):
                    nc.tensor.matmul(ps_sum[:, tc4:tc4 + 1],
                                     expT[0:LKV, tc4 * 128:(tc4 + 1) * 128],
                                     ones_col[0:LKV, :], start=True, stop=True)
                r_sb = xpool.tile([128, TOKT // 128], F32, tag="rsb")
                nc.vector.reciprocal(r_sb[:], ps_sum[:, 0:TOKT // 128])

                o_sb = opool.tile([128, TOKT // 128, D], F32, tag="osb")
                for tc4 in range(TOKT // 128):
                    for fh in range(2):
                        ps_o = pbig.tile([128, 512], F32, tag="ps")
                        nc.tensor.matmul(ps_o[:],
                                         expT[0:LKV, tc4 * 128:(tc4 + 1) * 128],
                                         e_sb[0:LKV, fh * 512:(fh + 1) * 512],
                                         start=True, stop=True)
                        nc.vector.tensor_scalar_mul(
                            o_sb[:, tc4, fh * 512:(fh + 1) * 512], ps_o[:],
                            r_sb[:, tc4:tc4 + 1])
                nc.gpsimd.dma_start(
                    o_d.ap()[b, t * TOKT:(t + 1) * TOKT, :]
                    .rearrange("(c p t) f -> p c t f", p=128, t=2),
                    o_sb[:])


_CACHE = {}


def kernel(x, y, Wq, bq, Wk, bk, Wv, bv, Wo, bo):
    global LAST_EXEC_TIME_NS, LAST_RESULTS
    x = np.ascontiguousarray(x, np.float32)
    y = np.ascontiguousarray(y, np.float32)
    use_bias = bool(np.any(bq) or np.any(bk) or np.any(bv) or np.any(bo))
    if use_bias not in _CACHE:
        _CACHE[use_bias] = _build(use_bias)
    nc = _CACHE[use_bias]

    shared = {
        "Wq": np.ascontiguousarray(Wq, np.float32),
        "Wk": np.ascontiguousarray(Wk, np.float32),
        "Wv": np.ascontiguousarray(Wv, np.float32),
        "Wo": np.ascontiguousarray(Wo, np.float32),
        "bq": np.ascontiguousarray(bq, np.float32),
        "bk": np.ascontiguousarray(bk, np.float32),
        "bv": np.ascontiguousarray(bv, np.float32),
        "bo": np.ascontiguousarray(bo, np.float32),
    }
    in_maps = []
    for i in range(N_CORES):
        m = dict(shared)
        m["x"] = np.ascontiguousarray(x[i * BPC:(i + 1) * BPC])
        m["y"] = np.ascontiguousarray(y[i * BPC:(i + 1) * BPC])
        in_maps.append(m)

    res = run_bass_kernel_spmd(nc, in_maps, core_ids=list(range(N_CORES)))
    LAST_EXEC_TIME_NS = res.exec_time_ns
    LAST_RESULTS = res
    return np.concatenate([res.results[i]["out"] for i in range(N_CORES)], axis=0)
